# revision 15
# baseline (speedup 1.0000x reference)
"""Grouped-Query Attention kernel for 8 Trainium2 NeuronCores.

Reference model: x[1,2048,2048] -> Q(32 heads x 64) / K,V(8 kv heads x 64),
per-head RMS-norm(Q,K) + RoPE, causal softmax attention, out-projection.

Sharding (tensor-parallel over heads): core c owns Q heads 4c..4c+3 and KV
head c (exactly its GQA group) and W_out rows [256c : 256c+256).  Each core
computes a full-shape partial output; the host sums the 8 partials (the
unshard step for a row-sharded W_out).

On-core strategy:
  - all matmul operands are bf16 (PE runs 1 cycle/row vs 4 for fp32);
    accumulation stays fp32 in PSUM, softmax statistics stay fp32
  - x and the weights are converted to bf16 on the host, so DMA traffic
    is halved and no on-chip conversion pass is needed
  - scores are built TRANSPOSED (S^T[j,i] = k_j . q_i) so PV needs no
    attention-matrix transpose and the softmax denominator comes free
    via an extra ones-column in V
  - RMS-norm of q/k bounds |scores/8| <= 8, so exp() without
    max-subtraction is safe; masked entries are zeroed after exp by
    multiplying with precomputed keep-masks (causal edge tiles dedupe
    to 4 patterns)
  - phase 2 runs i-block outer so denominators + out-projection for
    early token blocks overlap later attention; per (head, iblock) all
    QK matmuls issue before the PV accumulation so exp latency hides
  - q/k norm+rope are batched as 5 "heads" (4 q + 1 k) with the q/k
    scales and rotate-half signs folded into host-precomputed tables
"""

import numpy as np
import ml_dtypes

BF16 = ml_dtypes.bfloat16

T = 2048
D = 2048
NUM_HEADS = 32
NUM_KV = 8
HD = 64
N_CORES = 8
H_LOC = NUM_HEADS // N_CORES  # 4 q heads per core
EPS = 1e-6

TT = T // 128   # 16 t-tiles of 128 rows
CC = D // 128   # 16 contraction chunks
IBS = T // 512  # 4 i-blocks of 512 query positions
JBS = T // 128  # 16 j-blocks of 128 key positions

KEEP = "keep"
SKIP = "skip"
AFFINE = "affine"  # kept for test.py compat; no longer produced


def _classify_mask(mask: np.ndarray):
    """Per (ib, jb) scoresT tile: KEEP / SKIP / ('pat', idx) with deduped
    multiplicative keep-masks in S^T (j, i) layout.  A causal mask yields
    just 4 distinct edge patterns."""
    keep = ~mask
    status = [[KEEP] * JBS for _ in range(IBS)]
    pat_index: dict[bytes, int] = {}
    pats: list[np.ndarray] = []
    for ib in range(IBS):
        for jb in range(JBS):
            sub = keep[ib * 512:(ib + 1) * 512, jb * 128:(jb + 1) * 128]
            if sub.all():
                status[ib][jb] = KEEP
            elif not sub.any():
                status[ib][jb] = SKIP
            else:
                key = sub.tobytes()
                if key not in pat_index:
                    pat_index[key] = len(pats)
                    pats.append(sub.T.astype(np.float32))  # [128 j, 512 i]
                status[ib][jb] = ("pat", pat_index[key])
    patterns = (
        np.stack(pats) if pats else np.zeros((1, 128, 512), dtype=np.float32)
    )
    return status, patterns


def _split_multiwaits(nc):
    """walrus in this container accepts only ONE sync-wait per instruction;
    hoist extra waits onto preceding same-engine NoOps (program order on the
    engine queue preserves the gating)."""
    import bass_rust
    from concourse import mybir

    n_fixed = 0
    for fn in nc.m.functions:
        for bb in fn.blocks:
            out = []
            for ins in bb.instructions:
                si = ins.sync_info
                if si is not None and si.on_wait and len(si.on_wait) > 1:
                    waits = list(si.on_wait)
                    ups = list(si.on_update) if si.on_update else []
                    for k, w in enumerate(waits[:-1]):
                        nop = mybir.InstNoOp(
                            name=f"{ins.name}-wnop{k}", ins=[], outs=[]
                        )
                        nop.engine = ins.engine
                        nop.sync_info = bass_rust.SyncInfo(
                            on_wait=[w], on_update=[]
                        )
                        out.append(nop)
                    ins.sync_info = bass_rust.SyncInfo(
                        on_wait=[waits[-1]], on_update=ups
                    )
                    n_fixed += 1
                out.append(ins)
            bb.instructions = out
    return n_fixed


def _build_program(status, n_pat):
    import concourse.bass as bass
    import concourse.mybir as mybir
    import concourse.tile as tile
    from concourse.masks import make_identity

    f32 = mybir.dt.float32
    bf16 = mybir.dt.bfloat16
    AX = mybir.AxisListType
    AF = mybir.ActivationFunctionType

    nc = bass.Bass("TRN2", num_devices=N_CORES)
    x_d = nc.declare_dram_parameter("x", [T, D], bf16, isOutput=False)
    wqkv_d = nc.declare_dram_parameter(
        "wqkv", [D, H_LOC * HD + 2 * HD], bf16, isOutput=False
    )
    wo_d = nc.declare_dram_parameter("wo", [H_LOC * HD, D], bf16, isOutput=False)
    # combined 5-unit rope tables: 4 q heads + k, scales folded in
    cosa_d = nc.declare_dram_parameter("cosa", [T, 5 * HD], f32, isOutput=False)
    sina_d = nc.declare_dram_parameter("sina", [T, 5 * HD], f32, isOutput=False)
    mpat_d = nc.declare_dram_parameter(
        "mpat", [n_pat, 128, 512], bf16, isOutput=False
    )
    out_d = nc.declare_dram_parameter("out", [T, D], bf16, isOutput=True)

    NQKV = H_LOC * HD + 2 * HD  # 384: q heads, then k, then v
    NQK = (H_LOC + 1) * HD      # 320: q heads + k (norm/rope batch)

    def mmr(out, lhsT, rhs, **kw):
        nc.tensor.matmul(out, lhsT, rhs, **kw)

    with tile.TileContext(nc) as tc:
        with (
            tc.tile_pool(name="const", bufs=1) as const,
            tc.tile_pool(name="persist", bufs=1) as persist,
        ):
            ident = const.tile([128, 128], bf16)
            make_identity(nc, ident)
            eps_t = const.tile([128, 1], f32)
            nc.vector.memset(eps_t, EPS)

            # persistent across phases (all bf16 matmul operands).
            # qT/kT hold only the real 64 head dims: score matmuls
            # contract K=64, so no zero-padding rows are needed.
            qkT = persist.tile([64, 5, T], bf16)
            # v with aux columns:
            #  v_aug  [128,TT,65]:  cols 0:64 = v, col 64 = 1  (even head of pair)
            #  v_aug2 [128,TT,128]: col 32 = 1, cols 64:128 = v (odd head of pair)
            v_aug = persist.tile([128, TT, 65], bf16)
            nc.vector.memset(v_aug[:, :, 64:65], 1.0)
            v_aug2 = persist.tile([128, TT, 128], bf16)
            nc.vector.memset(v_aug2[:, :, 0:64], 0.0)
            nc.vector.memset(v_aug2[:, :, 32:33], 1.0)
            ctxB = [persist.tile([128, T], bf16, name=f"ctxB{p}") for p in range(2)]
            dbc = [persist.tile([128, T], f32, name=f"dbc{p}") for p in range(2)]

            # ---------- phase 1: transpose x, project qkv, norm+rope ----------
            with (
                tc.tile_pool(name="p1w", bufs=1) as p1w,
                tc.tile_pool(name="p1s", bufs=2) as p1s,
                tc.tile_pool(name="p1t", bufs=2) as p1t,
                tc.tile_pool(name="ps1a", bufs=2, space="PSUM") as ps1a,
                tc.tile_pool(name="ps1b", bufs=2, space="PSUM") as ps1b,
                tc.tile_pool(name="ps1c", bufs=1, space="PSUM") as ps1c,
            ):
                # weight/table DMAs ride the DVE trigger queue so the
                # per-tt x loads on the sync queue start immediately
                wqkv_sb = p1w.tile([128, CC, NQKV], bf16)
                nc.scalar.dma_start(
                    out=wqkv_sb, in_=wqkv_d.rearrange("(cc p) m -> p cc m", p=128)
                )
                ctab = p1w.tile([128, TT, 5, HD], f32, name="ctab")
                nc.scalar.dma_start(
                    out=ctab,
                    in_=cosa_d.rearrange("(tt p) (u d) -> p tt u d", p=128, u=5),
                )
                stab = p1w.tile([128, TT, 5, HD], f32, name="stab")
                nc.scalar.dma_start(
                    out=stab,
                    in_=sina_d.rearrange("(tt p) (u d) -> p tt u d", p=128, u=5),
                )

                for tt in range(TT):
                    x_nat = p1s.tile([128, D], bf16, tag="x_nat")
                    nc.sync.dma_start(
                        out=x_nat, in_=x_d[tt * 128:(tt + 1) * 128, :]
                    )
                    xt_col = p1s.tile([128, CC, 128], bf16, tag="xt_col")
                    for cg in range(4):
                        pst = ps1a.tile([128, 512], bf16, tag="pst")
                        for k in range(4):
                            cc = cg * 4 + k
                            nc.tensor.transpose(
                                pst[:, k * 128:(k + 1) * 128],
                                x_nat[:, cc * 128:(cc + 1) * 128],
                                ident,
                            )
                        eng = nc.vector.tensor_copy if cg % 2 == 0 else nc.scalar.copy
                        eng(
                            xt_col[:, cg * 4:(cg + 1) * 4, :]
                            .rearrange("p a b -> p (a b)"),
                            pst,
                        )
                    psqkv = ps1b.tile([128, NQKV], f32, tag="psqkv")
                    for cc in range(CC):
                        mmr(psqkv, xt_col[:, cc, :], wqkv_sb[:, cc, :],
                            start=(cc == 0), stop=(cc == CC - 1))
                    psv = psqkv[:, NQK:NQKV]

                    nc.scalar.copy(v_aug[:, tt, 0:64], psv)
                    nc.scalar.copy(v_aug2[:, tt, 64:128], psv)

                    # rms-norm + rope for 4 q heads + k in one 5-unit batch
                    # (PSUM -> SBUF first: DVE tensor-tensor can't read PSUM)
                    qk5 = p1t.tile([128, 5, HD], f32, tag="qk5")
                    nc.scalar.copy(
                        qk5, psqkv[:, 0:NQK].rearrange("p (u d) -> p u d", u=5)
                    )
                    sq = p1t.tile([128, 5, HD], f32, tag="sq")
                    nc.gpsimd.tensor_mul(sq, qk5, qk5)
                    ssum = p1t.tile([128, 5, 1], f32, tag="ssum")
                    nc.vector.reduce_sum(ssum, sq, axis=AX.X)
                    rinv = p1t.tile([128, 5, 1], f32, tag="rinv")
                    nc.scalar.activation(rinv, ssum, AF.Sqrt,
                                         bias=eps_t[:, 0:1], scale=1.0 / HD)
                    nc.vector.reciprocal(rinv, rinv)
                    qn = p1t.tile([128, 5, HD], f32, tag="qn")
                    nc.vector.tensor_mul(
                        qn, qk5, rinv.to_broadcast([128, 5, HD])
                    )
                    qr = p1t.tile([128, 5, HD], f32, tag="qr")
                    nc.vector.tensor_mul(qr, qn, ctab[:, tt, :, :])
                    qrot = p1t.tile([128, 5, HD], f32, tag="qrot")
                    nc.gpsimd.tensor_mul(
                        qrot[:, :, 0:32], qn[:, :, 32:64],
                        stab[:, tt, :, 0:32],
                    )
                    nc.gpsimd.tensor_mul(
                        qrot[:, :, 32:64], qn[:, :, 0:32],
                        stab[:, tt, :, 32:64],
                    )
                    qb = p1t.tile([128, 5, HD], bf16, tag="qb")
                    nc.gpsimd.tensor_add(qb, qr, qrot)

                    # transpose the 5 units into qT / kT
                    psqt = ps1c.tile([64, 5, 128], bf16, tag="psqt")
                    for u in range(5):
                        nc.tensor.transpose(psqt[:, u, :], qb[:, u, :], ident)
                    nc.vector.tensor_copy(
                        qkT[:, :, tt * 128:(tt + 1) * 128], psqt
                    )

            # ---------- phase 2: attention + denominators + out-proj ----------
            with (
                tc.tile_pool(name="p2w", bufs=1) as p2w,
                tc.tile_pool(name="p2e", bufs=8) as p2e,
                tc.tile_pool(name="ps2s", bufs=4, space="PSUM") as ps2s,
                tc.tile_pool(name="ps2c", bufs=2, space="PSUM") as ps2c,
                tc.tile_pool(name="ps2o", bufs=2, space="PSUM") as ps2o,
                tc.tile_pool(name="p2d", bufs=4, space="DRAM") as p2d,
            ):
                wo_sb = [p2w.tile([128, D], bf16, name=f"wo{p}") for p in range(2)]
                for p in range(2):
                    nc.scalar.dma_start(
                        out=wo_sb[p], in_=wo_d[p * 128:(p + 1) * 128, :]
                    )
                mpat_sb = p2w.tile([128, n_pat, 512], bf16)
                nc.scalar.dma_start(
                    out=mpat_sb, in_=mpat_d.rearrange("n p f -> p n f")
                )

                def bcast64(sl):
                    return bass.AP(
                        tensor=sl.tensor, offset=sl.offset,
                        ap=[[0, 64], [1, 512]],
                    )

                inv_sqrt_d = float(1.0 / np.sqrt(HD))

                def out_proj(ib):
                    for t4 in range(4):
                        tt = ib * 4 + t4
                        for cb in range(4):
                            pso = ps2o.tile([128, 512], f32, tag="pso")
                            for pair in range(2):
                                mmr(pso,
                                    ctxB[pair][:, tt * 128:(tt + 1) * 128],
                                    wo_sb[pair][:, cb * 512:(cb + 1) * 512],
                                    start=(pair == 0), stop=(pair == 1))
                            ot = p2e.tile([128, 512], bf16, tag="ot")
                            nc.vector.tensor_copy(ot, pso)
                            nc.sync.dma_start(
                                out=out_d[tt * 128:(tt + 1) * 128,
                                          cb * 512:(cb + 1) * 512],
                                in_=ot,
                            )

                for ib in range(IBS):
                    iw = slice(ib * 512, (ib + 1) * 512)
                    jbs = [jb for jb in range(JBS) if status[ib][jb] != SKIP]
                    psc_of = {}
                    for h in range(H_LOC):
                        pair, sub = divmod(h, 2)
                        psc = ps2c.tile([128, 512], f32, tag="psc")
                        psc_of[h] = psc
                        if sub == 0:
                            ctx_out = psc[0:65, :]
                            lhs_of = lambda jb: v_aug[:, jb, :]
                        else:
                            ctx_out = psc
                            lhs_of = lambda jb: v_aug2[:, jb, :]
                        # all QK matmuls first; exp/mask trail on ACT/DVE;
                        # then the PV accumulation chain (never stalls PE)
                        ets = []
                        for jb in jbs:
                            pss = ps2s.tile([128, 512], f32, tag="pss")
                            mmr(pss, qkT[:, 4, jb * 128:(jb + 1) * 128],
                                qkT[:, h, iw], start=True, stop=True)
                            et = p2e.tile([128, 512], bf16, tag="et")
                            nc.scalar.activation(et, pss, AF.Exp,
                                                 scale=inv_sqrt_d)
                            st = status[ib][jb]
                            if isinstance(st, tuple):
                                nc.vector.tensor_mul(
                                    et, et, mpat_sb[:, st[1], :]
                                )
                            ets.append(et)
                        for n, jb in enumerate(jbs):
                            mmr(ctx_out, lhs_of(jb), ets[n],
                                start=(n == 0), stop=(n == len(jbs) - 1))

                    # denominators: reciprocal into SBUF staging (same
                    # partition rows), bounce via DRAM so the partition-
                    # broadcast read (step-0 partition AP) has a DRAM
                    # source, then scale ctx into bf16 ctxB
                    dscr = p2d.tile([2, 2, 512], f32, tag="dscr")
                    den_sb = p2e.tile([65, 2, 512], f32, tag="den_sb")
                    for pair in range(2):
                        pe, po = psc_of[2 * pair], psc_of[2 * pair + 1]
                        nc.vector.reciprocal(den_sb[64:65, pair, :],
                                             pe[64:65, :])
                        nc.vector.reciprocal(den_sb[32:33, pair, :],
                                             po[32:33, :])
                        nc.sync.dma_start(out=dscr[0:1, pair, :],
                                          in_=den_sb[64:65, pair, :])
                        nc.sync.dma_start(out=dscr[1:2, pair, :],
                                          in_=den_sb[32:33, pair, :])
                        nc.gpsimd.dma_start(
                            out=dbc[pair][0:64, iw],
                            in_=bcast64(dscr[0, pair:pair + 1, :]),
                        )
                        nc.gpsimd.dma_start(
                            out=dbc[pair][64:128, iw],
                            in_=bcast64(dscr[1, pair:pair + 1, :]),
                        )
                        # stage ctx PSUM -> SBUF (only DVE/ACT read PSUM),
                        # then scale by 1/den on DVE into bf16
                        ctx_s = p2e.tile([128, 512], f32, tag="ctx_s")
                        nc.vector.tensor_copy(ctx_s[0:64, :], pe[0:64, :])
                        nc.vector.tensor_copy(ctx_s[64:128, :], po[64:128, :])
                        nc.vector.tensor_mul(
                            ctxB[pair][:, iw], ctx_s, dbc[pair][:, iw],
                        )

                    # out-projection lags one i-block behind attention so
                    # the 1/den DMA round-trip hides under the next block
                    if ib > 0:
                        out_proj(ib - 1)
                out_proj(IBS - 1)

    _split_multiwaits(nc)
    return nc


_CACHE = {}


def _get_program(mask_key, status, n_pat):
    if mask_key not in _CACHE:
        _CACHE[mask_key] = _build_program(status, n_pat)
    return _CACHE[mask_key]


def _prepare(x, mask, cos, sin, W_query, W_key, W_value, W_out,
             q_scale, k_scale):
    """Host-side prep: fold scales into rope tables, shard weights,
    classify the mask.  Returns (nc, in_maps)."""
    cos = np.asarray(cos, dtype=np.float32)
    sin = np.asarray(sin, dtype=np.float32)
    W_query = np.asarray(W_query, dtype=np.float32)
    W_key = np.asarray(W_key, dtype=np.float32)
    W_value = np.asarray(W_value, dtype=np.float32)
    W_out = np.asarray(W_out, dtype=np.float32)
    q_scale = np.asarray(q_scale, dtype=np.float32)
    k_scale = np.asarray(k_scale, dtype=np.float32)
    mask = np.asarray(mask)

    xf = np.ascontiguousarray(
        np.asarray(x).reshape(T, D).astype(BF16)
    )

    # rope = qn*cos' + shuffle32(qn)*sin' with the rotate-half signs and the
    # post-norm q/k scales folded into the tables:
    #   rope(s*qn) = qn*(s*cos) + shuffle32(qn)*(shuffle32(s)*sin+-)
    def tables(scale):
        perm = np.concatenate([scale[HD // 2:], scale[:HD // 2]])
        c = (cos * scale[None, :]).astype(np.float32)
        s = (sin * perm[None, :]).astype(np.float32)
        s[:, :HD // 2] *= -1.0
        return c, s

    cq, sq_t = tables(q_scale)
    ck, sk_t = tables(k_scale)
    # 5-unit tables: 4 q heads then k  -> [T, 320]
    cosa = np.ascontiguousarray(
        np.concatenate([cq, cq, cq, cq, ck], axis=1)
    )
    sina = np.ascontiguousarray(
        np.concatenate([sq_t, sq_t, sq_t, sq_t, sk_t], axis=1)
    )

    status, patterns = _classify_mask(mask)
    nc = _get_program(mask.tobytes(), status, patterns.shape[0])
    patterns_bf = patterns.astype(BF16)

    in_maps = []
    for c in range(N_CORES):
        qcols = slice(c * H_LOC * HD, (c + 1) * H_LOC * HD)
        kvcols = slice(c * HD, (c + 1) * HD)
        wqkv = np.concatenate(
            [W_query[:, qcols], W_key[:, kvcols], W_value[:, kvcols]], axis=1
        ).astype(BF16)
        in_maps.append({
            "x": xf,
            "wqkv": np.ascontiguousarray(wqkv),
            "wo": np.ascontiguousarray(W_out[qcols, :].astype(BF16)),
            "cosa": cosa, "sina": sina,
            "mpat": patterns_bf,
        })
    return nc, in_maps


def kernel(x, mask, cos, sin, W_query, W_key, W_value, W_out,
           q_scale, k_scale):
    out_dtype = np.asarray(x).dtype
    nc, in_maps = _prepare(x, mask, cos, sin, W_query, W_key, W_value,
                           W_out, q_scale, k_scale)

    from concourse.bass_utils import run_bass_kernel_spmd

    res = run_bass_kernel_spmd(nc, in_maps, list(range(N_CORES)))
    acc = res.results[0]["out"].astype(np.float32)
    for c in range(1, N_CORES):
        acc = acc + res.results[c]["out"].astype(np.float32)
    return acc.reshape(1, T, D).astype(out_dtype)


# revision 40
# speedup vs baseline: 407.4481x; 407.4481x over previous
"""Grouped-Query Attention kernel for 8 Trainium2 NeuronCores.

Reference model: x[1,2048,2048] -> Q(32 heads x 64) / K,V(8 kv heads x 64),
per-head RMS-norm(Q,K) + RoPE, causal softmax attention, out-projection.

Sharding (tensor-parallel over heads): core c owns Q heads 4c..4c+3 and KV
head c (exactly its GQA group) and W_out rows [256c : 256c+256).  Each core
computes a full-shape partial output; the host sums the 8 partials (the
unshard step for a row-sharded W_out).

On-core strategy:
  - all matmul operands are bf16 (PE runs 1 cycle/row vs 4 for fp32);
    accumulation stays fp32 in PSUM, softmax statistics stay fp32
  - x and the weights are converted to bf16 on the host, so DMA traffic
    is halved and no on-chip conversion pass is needed
  - scores are built TRANSPOSED (S^T[j,i] = k_j . q_i) so PV needs no
    attention-matrix transpose and the softmax denominator comes free
    via an extra ones-column in V
  - RMS-norm of q/k bounds |scores/8| <= 8, so exp() without
    max-subtraction is safe; masked entries are zeroed after exp by
    multiplying with precomputed keep-masks (causal edge tiles dedupe
    to 4 patterns)
  - phase 2 runs i-block outer so denominators + out-projection for
    early token blocks overlap later attention; per (head, iblock) all
    QK matmuls issue before the PV accumulation so exp latency hides
  - q/k norm+rope are batched as 5 "heads" (4 q + 1 k) with the q/k
    scales and rotate-half signs folded into host-precomputed tables
"""

import numpy as np
import ml_dtypes

BF16 = ml_dtypes.bfloat16

T = 2048
D = 2048
NUM_HEADS = 32
NUM_KV = 8
HD = 64
N_CORES = 8
H_LOC = NUM_HEADS // N_CORES  # 4 q heads per core
EPS = 1e-6

TT = T // 128   # 16 t-tiles of 128 rows
CC = D // 128   # 16 contraction chunks
IBS = T // 512  # 4 i-blocks of 512 query positions
JBS = T // 128  # 16 j-blocks of 128 key positions

KEEP = "keep"
SKIP = "skip"
AFFINE = "affine"  # kept for test.py compat; no longer produced


def _classify_mask(mask: np.ndarray):
    """Per (ib, jb) scoresT tile: KEEP / SKIP / ('pat', idx) with deduped
    multiplicative keep-masks in S^T (j, i) layout.  A causal mask yields
    just 4 distinct edge patterns."""
    keep = ~mask
    status = [[KEEP] * JBS for _ in range(IBS)]
    pat_index: dict[bytes, int] = {}
    pats: list[np.ndarray] = []
    for ib in range(IBS):
        for jb in range(JBS):
            sub = keep[ib * 512:(ib + 1) * 512, jb * 128:(jb + 1) * 128]
            if sub.all():
                status[ib][jb] = KEEP
            elif not sub.any():
                status[ib][jb] = SKIP
            else:
                key = sub.tobytes()
                if key not in pat_index:
                    pat_index[key] = len(pats)
                    pats.append(sub.T.astype(np.float32))  # [128 j, 512 i]
                status[ib][jb] = ("pat", pat_index[key])
    patterns = (
        np.stack(pats) if pats else np.zeros((1, 128, 512), dtype=np.float32)
    )
    return status, patterns


def _split_multiwaits(nc):
    """walrus in this container accepts only ONE sync-wait per instruction;
    hoist extra waits onto preceding same-engine NoOps (program order on the
    engine queue preserves the gating)."""
    import bass_rust
    from concourse import mybir

    n_fixed = 0
    for fn in nc.m.functions:
        for bb in fn.blocks:
            out = []
            for ins in bb.instructions:
                si = ins.sync_info
                if si is not None and si.on_wait and len(si.on_wait) > 1:
                    waits = list(si.on_wait)
                    ups = list(si.on_update) if si.on_update else []
                    for k, w in enumerate(waits[:-1]):
                        nop = mybir.InstNoOp(
                            name=f"{ins.name}-wnop{k}", ins=[], outs=[]
                        )
                        nop.engine = ins.engine
                        nop.sync_info = bass_rust.SyncInfo(
                            on_wait=[w], on_update=[]
                        )
                        out.append(nop)
                    ins.sync_info = bass_rust.SyncInfo(
                        on_wait=[waits[-1]], on_update=ups
                    )
                    n_fixed += 1
                out.append(ins)
            bb.instructions = out
    return n_fixed


def _build_program(status, n_pat):
    import concourse.bass as bass
    import concourse.mybir as mybir
    import concourse.tile as tile
    from concourse.masks import make_identity

    f32 = mybir.dt.float32
    bf16 = mybir.dt.bfloat16
    AX = mybir.AxisListType
    AF = mybir.ActivationFunctionType

    nc = bass.Bass("TRN2", num_devices=N_CORES)
    x_d = nc.declare_dram_parameter("x", [T, D], bf16, isOutput=False)
    wqkv_d = nc.declare_dram_parameter(
        "wqkv", [D, H_LOC * HD + 2 * HD], bf16, isOutput=False
    )
    wo_d = nc.declare_dram_parameter("wo", [H_LOC * HD, D], bf16, isOutput=False)
    # combined 5-unit rope tables: 4 q heads + k, scales folded in
    cosa_d = nc.declare_dram_parameter("cosa", [T, 5 * HD], bf16, isOutput=False)
    sina_d = nc.declare_dram_parameter("sina", [T, 5 * HD], bf16, isOutput=False)
    mpat_d = nc.declare_dram_parameter(
        "mpat", [n_pat, 128, 512], bf16, isOutput=False
    )
    out_d = nc.declare_dram_parameter("out", [T, D], bf16, isOutput=True)

    NQKV = H_LOC * HD + 2 * HD  # 384: q heads, then k, then v
    NQK = (H_LOC + 1) * HD      # 320: q heads + k (norm/rope batch)

    def mmr(out, lhsT, rhs, **kw):
        nc.tensor.matmul(out, lhsT, rhs, **kw)

    with tile.TileContext(nc) as tc:
        with (
            tc.tile_pool(name="const", bufs=1) as const,
            tc.tile_pool(name="persist", bufs=1) as persist,
        ):
            ident = const.tile([128, 128], bf16)
            make_identity(nc, ident)
            eps_t = const.tile([128, 1], f32)
            nc.vector.memset(eps_t, EPS)
            ones_t = const.tile([128, 64], bf16)
            nc.vector.memset(ones_t, 1.0)

            # persistent across phases (all bf16 matmul operands).
            # qT/kT hold only the real 64 head dims: score matmuls
            # contract K=64, so no zero-padding rows are needed.
            qkT = persist.tile([64, 5, T], bf16)
            # v with aux columns:
            #  v_aug  [128,TT,65]:  cols 0:64 = v, col 64 = 1  (even head of pair)
            #  v_aug2 [128,TT,128]: col 32 = 1, cols 64:128 = v (odd head of pair)
            v_aug = persist.tile([128, TT, 65], bf16)
            nc.vector.memset(v_aug[:, :, 64:65], 1.0)
            v_aug2 = persist.tile([128, TT, 128], bf16)
            nc.vector.memset(v_aug2[:, :, 0:64], 0.0)
            nc.vector.memset(v_aug2[:, :, 32:33], 1.0)
            ctxB = [persist.tile([128, T], bf16, name=f"ctxB{p}") for p in range(2)]
            dbc = [persist.tile([128, T], f32, name=f"dbc{p}") for p in range(2)]
            wo_sb = [persist.tile([128, D], bf16, name=f"wo{p}") for p in range(2)]
            for p in range(2):
                nc.gpsimd.dma_start(
                    out=wo_sb[p], in_=wo_d[p * 128:(p + 1) * 128, :]
                )
            mpat_sb = persist.tile([128, n_pat, 512], bf16, name="mpat_sb")
            nc.gpsimd.dma_start(
                out=mpat_sb, in_=mpat_d.rearrange("n p f -> p n f")
            )

            # ---------- phase 1: transpose x, project qkv, norm+rope ----------
            with (
                tc.tile_pool(name="p1w", bufs=1) as p1w,
                tc.tile_pool(name="p1s", bufs=3) as p1s,
                tc.tile_pool(name="p1t", bufs=3) as p1t,
                tc.tile_pool(name="ps1a", bufs=3, space="PSUM") as ps1a,
                tc.tile_pool(name="ps1b", bufs=2, space="PSUM") as ps1b,
                tc.tile_pool(name="ps1c", bufs=2, space="PSUM") as ps1c,
            ):
                # weight/table DMAs ride the DVE trigger queue so the
                # per-tt x loads on the sync queue start immediately
                wqkv_sb = p1w.tile([128, CC, NQKV], bf16)
                wqkv_r = wqkv_d.rearrange("(cc p) m -> p cc m", p=128)
                ctab = p1w.tile([128, TT, 5, HD], bf16, name="ctab")
                ctab_r = cosa_d.rearrange("(tt p) (u d) -> p tt u d", p=128, u=5)
                stab = p1w.tile([128, TT, 5, HD], bf16, name="stab")
                stab_r = sina_d.rearrange("(tt p) (u d) -> p tt u d", p=128, u=5)
                # chunked + interleaved so the first tiles' operands land
                # early instead of queueing behind 2.6 MB of tables
                for wc in range(4):
                    sl = slice(wc * 4, (wc + 1) * 4)
                    nc.scalar.dma_start(out=wqkv_sb[:, sl, :],
                                        in_=wqkv_r[:, sl, :])
                    nc.scalar.dma_start(out=ctab[:, sl], in_=ctab_r[:, sl])
                    nc.scalar.dma_start(out=stab[:, sl], in_=stab_r[:, sl])

                for tt in range(TT):
                    x_nat = p1s.tile([128, D], bf16, tag="x_nat")
                    nc.sync.dma_start(
                        out=x_nat, in_=x_d[tt * 128:(tt + 1) * 128, :]
                    )
                    xt_col = p1s.tile([128, CC, 128], bf16, tag="xt_col")
                    for cg in range(4):
                        pst = ps1a.tile([128, 512], bf16, tag="pst")
                        for k in range(4):
                            cc = cg * 4 + k
                            nc.tensor.transpose(
                                pst[:, k * 128:(k + 1) * 128],
                                x_nat[:, cc * 128:(cc + 1) * 128],
                                ident,
                            )
                        eng = nc.vector.tensor_copy if cg % 2 == 0 else nc.scalar.copy
                        eng(
                            xt_col[:, cg * 4:(cg + 1) * 4, :]
                            .rearrange("p a b -> p (a b)"),
                            pst,
                        )
                    psqkv = ps1b.tile([128, NQKV], f32, tag="psqkv")
                    for cc in range(CC):
                        mmr(psqkv, xt_col[:, cc, :], wqkv_sb[:, cc, :],
                            start=(cc == 0), stop=(cc == CC - 1))
                    psv = psqkv[:, NQK:NQKV]

                    nc.scalar.copy(v_aug[:, tt, 0:64], psv)
                    nc.scalar.copy(v_aug2[:, tt, 64:128], psv)

                    # rms-norm + rope for 4 q heads + k in one 5-unit batch
                    # (PSUM -> SBUF first: DVE tensor-tensor can't read PSUM)
                    qk5 = p1t.tile([128, 5, HD], f32, tag="qk5")
                    nc.scalar.copy(
                        qk5, psqkv[:, 0:NQK].rearrange("p (u d) -> p u d", u=5)
                    )
                    sq = p1t.tile([128, 5, HD], f32, tag="sq")
                    nc.scalar.activation(
                        sq, psqkv[:, 0:NQK].rearrange("p (u d) -> p u d", u=5),
                        AF.Square,
                    )
                    ssum = p1t.tile([128, 5, 1], f32, tag="ssum")
                    nc.vector.reduce_sum(ssum, sq, axis=AX.X)
                    rinv = p1t.tile([128, 5, 1], f32, tag="rinv")
                    nc.scalar.activation(rinv, ssum, AF.Sqrt,
                                         bias=eps_t[:, 0:1], scale=1.0 / HD)
                    nc.vector.reciprocal(rinv, rinv)
                    qn = p1t.tile([128, 5, HD], bf16, tag="qn")
                    nc.vector.tensor_mul(
                        qn, qk5, rinv.to_broadcast([128, 5, HD])
                    )
                    qr = p1t.tile([128, 5, HD], bf16, tag="qr")
                    nc.vector.tensor_mul(qr, qn, ctab[:, tt, :, :])
                    qrot = p1t.tile([128, 5, HD], bf16, tag="qrot")
                    nc.gpsimd.tensor_mul(
                        qrot[:, :, 0:32], qn[:, :, 32:64],
                        stab[:, tt, :, 0:32],
                    )
                    nc.gpsimd.tensor_mul(
                        qrot[:, :, 32:64], qn[:, :, 0:32],
                        stab[:, tt, :, 32:64],
                    )
                    qb = p1t.tile([128, 5, HD], bf16, tag="qb")
                    nc.vector.tensor_add(qb, qr, qrot)

                    # transpose the 5 units into qT / kT
                    psqt = ps1c.tile([64, 5, 128], bf16, tag="psqt")
                    for u in range(5):
                        nc.tensor.transpose(psqt[:, u, :], qb[:, u, :], ident)
                    nc.vector.tensor_copy(
                        qkT[:, :, tt * 128:(tt + 1) * 128], psqt
                    )

            # ---------- phase 2: attention + denominators + out-proj ----------
            with (
                tc.tile_pool(name="p2e", bufs=8) as p2e,
                tc.tile_pool(name="ps2s", bufs=4, space="PSUM") as ps2s,
                tc.tile_pool(name="ps2c", bufs=2, space="PSUM") as ps2c,
                tc.tile_pool(name="ps2o", bufs=2, space="PSUM") as ps2o,
            ):

                inv_sqrt_d = float(1.0 / np.sqrt(HD))

                def out_proj_quarter(ib, t4):
                    tt = ib * 4 + t4
                    for cb in range(4):
                        pso = ps2o.tile([128, 512], f32, tag="pso")
                        for pair in range(2):
                            mmr(pso,
                                ctxB[pair][:, tt * 128:(tt + 1) * 128],
                                wo_sb[pair][:, cb * 512:(cb + 1) * 512],
                                start=(pair == 0), stop=(pair == 1))
                        ot = p2e.tile([128, 512], bf16, tag="ot")
                        nc.vector.tensor_copy(ot, pso)
                        nc.sync.dma_start(
                            out=out_d[tt * 128:(tt + 1) * 128,
                                      cb * 512:(cb + 1) * 512],
                            in_=ot,
                        )

                def den_pair(ib, pair, pe, po):
                    # denominators: reciprocal (bf16) into SBUF staging at
                    # the same partition rows, then broadcast across the
                    # partition dim with a K=1 ones-matmul (out reuses a
                    # pss ring slot), stage to SBUF, scale ctx into ctxB
                    iw = slice(ib * 512, (ib + 1) * 512)
                    den_sb = p2e.tile([65, 512], bf16, tag="den_sb")
                    with nc.allow_low_precision(
                        reason="1/den in bf16: 0.4% on softmax scale is "
                               "well inside the 2e-2 tolerance"
                    ):
                        nc.vector.reciprocal(den_sb[64:65, :], pe[64:65, :])
                        nc.vector.reciprocal(den_sb[32:33, :], po[32:33, :])
                    pdb = ps2s.tile([128, 512], f32, tag="pss")
                    mmr(pdb[0:64, :], ones_t[64:65, :], den_sb[64:65, :],
                        start=True, stop=True)
                    mmr(pdb[64:128, :], ones_t[32:33, :], den_sb[32:33, :],
                        start=True, stop=True)
                    nc.vector.tensor_copy(dbc[pair][:, iw], pdb)
                    # stage ctx PSUM -> SBUF (only DVE/ACT read PSUM),
                    # then scale by 1/den on DVE into bf16
                    ctx_s = p2e.tile([128, 512], f32, tag="ctx_s")
                    nc.vector.tensor_copy(ctx_s[0:64, :], pe[0:64, :])
                    nc.vector.tensor_copy(ctx_s[64:128, :], po[64:128, :])
                    nc.vector.tensor_mul(
                        ctxB[pair][:, iw], ctx_s, dbc[pair][:, iw],
                    )

                for ib in range(IBS):
                    iw = slice(ib * 512, (ib + 1) * 512)
                    jbs = [jb for jb in range(JBS) if status[ib][jb] != SKIP]
                    psc_of = {}
                    for h in range(H_LOC):
                        pair, sub = divmod(h, 2)
                        psc = ps2c.tile([128, 512], f32, tag="psc")
                        psc_of[h] = psc
                        if sub == 0:
                            ctx_out = psc[0:65, :]
                            lhs_of = lambda jb: v_aug[:, jb, :]
                        else:
                            ctx_out = psc
                            lhs_of = lambda jb: v_aug2[:, jb, :]
                        # all QK matmuls first; exp/mask trail on ACT/DVE;
                        # then the PV accumulation chain (never stalls PE)
                        ets = []
                        for jb in jbs:
                            pss = ps2s.tile([128, 512], f32, tag="pss")
                            mmr(pss, qkT[:, 4, jb * 128:(jb + 1) * 128],
                                qkT[:, h, iw], start=True, stop=True)
                            et = p2e.tile([128, 512], bf16, tag="et")
                            nc.scalar.activation(et, pss, AF.Exp,
                                                 scale=inv_sqrt_d)
                            st = status[ib][jb]
                            if isinstance(st, tuple):
                                nc.vector.tensor_mul(
                                    et, et, mpat_sb[:, st[1], :]
                                )
                            ets.append(et)
                        for n, jb in enumerate(jbs):
                            mmr(ctx_out, lhs_of(jb), ets[n],
                                start=(n == 0), stop=(n == len(jbs) - 1))
                        # denominator bounce starts as soon as a pair's
                        # two heads are done, overlapping the next heads
                        if sub == 1:
                            den_pair(ib, pair, psc_of[h - 1], psc)
                            # out-projection lags one i-block behind; its
                            # matmuls fill PE under this block's exp waits
                            if ib > 0:
                                out_proj_quarter(ib - 1, 2 * pair)
                                out_proj_quarter(ib - 1, 2 * pair + 1)
                for t4 in range(4):
                    out_proj_quarter(IBS - 1, t4)

    _split_multiwaits(nc)
    return nc


_CACHE = {}


def _get_program(mask_key, status, n_pat):
    if mask_key not in _CACHE:
        _CACHE[mask_key] = _build_program(status, n_pat)
    return _CACHE[mask_key]


def _prepare(x, mask, cos, sin, W_query, W_key, W_value, W_out,
             q_scale, k_scale):
    """Host-side prep: fold scales into rope tables, shard weights,
    classify the mask.  Returns (nc, in_maps)."""
    cos = np.asarray(cos, dtype=np.float32)
    sin = np.asarray(sin, dtype=np.float32)
    W_query = np.asarray(W_query, dtype=np.float32)
    W_key = np.asarray(W_key, dtype=np.float32)
    W_value = np.asarray(W_value, dtype=np.float32)
    W_out = np.asarray(W_out, dtype=np.float32)
    q_scale = np.asarray(q_scale, dtype=np.float32)
    k_scale = np.asarray(k_scale, dtype=np.float32)
    mask = np.asarray(mask)

    xf = np.ascontiguousarray(
        np.asarray(x).reshape(T, D).astype(BF16)
    )

    # rope = qn*cos' + shuffle32(qn)*sin' with the rotate-half signs and the
    # post-norm q/k scales folded into the tables:
    #   rope(s*qn) = qn*(s*cos) + shuffle32(qn)*(shuffle32(s)*sin+-)
    def tables(scale):
        perm = np.concatenate([scale[HD // 2:], scale[:HD // 2]])
        c = (cos * scale[None, :]).astype(np.float32)
        s = (sin * perm[None, :]).astype(np.float32)
        s[:, :HD // 2] *= -1.0
        return c, s

    cq, sq_t = tables(q_scale)
    ck, sk_t = tables(k_scale)
    # 5-unit tables: 4 q heads then k  -> [T, 320]
    cosa = np.ascontiguousarray(
        np.concatenate([cq, cq, cq, cq, ck], axis=1).astype(BF16)
    )
    sina = np.ascontiguousarray(
        np.concatenate([sq_t, sq_t, sq_t, sq_t, sk_t], axis=1).astype(BF16)
    )

    status, patterns = _classify_mask(mask)
    nc = _get_program(mask.tobytes(), status, patterns.shape[0])
    patterns_bf = patterns.astype(BF16)

    in_maps = []
    for c in range(N_CORES):
        qcols = slice(c * H_LOC * HD, (c + 1) * H_LOC * HD)
        kvcols = slice(c * HD, (c + 1) * HD)
        wqkv = np.concatenate(
            [W_query[:, qcols], W_key[:, kvcols], W_value[:, kvcols]], axis=1
        ).astype(BF16)
        in_maps.append({
            "x": xf,
            "wqkv": np.ascontiguousarray(wqkv),
            "wo": np.ascontiguousarray(W_out[qcols, :].astype(BF16)),
            "cosa": cosa, "sina": sina,
            "mpat": patterns_bf,
        })
    return nc, in_maps


def kernel(x, mask, cos, sin, W_query, W_key, W_value, W_out,
           q_scale, k_scale):
    out_dtype = np.asarray(x).dtype
    nc, in_maps = _prepare(x, mask, cos, sin, W_query, W_key, W_value,
                           W_out, q_scale, k_scale)

    from concourse.bass_utils import run_bass_kernel_spmd

    res = run_bass_kernel_spmd(nc, in_maps, list(range(N_CORES)))
    acc = res.results[0]["out"].astype(np.float32)
    for c in range(1, N_CORES):
        acc = acc + res.results[c]["out"].astype(np.float32)
    return acc.reshape(1, T, D).astype(out_dtype)


# revision 46
# speedup vs baseline: 424.8163x; 1.0426x over previous
"""Grouped-Query Attention kernel for 8 Trainium2 NeuronCores.

Reference model: x[1,2048,2048] -> Q(32 heads x 64) / K,V(8 kv heads x 64),
per-head RMS-norm(Q,K) + RoPE, causal softmax attention, out-projection.

Sharding (tensor-parallel over heads): core c owns Q heads 4c..4c+3 and KV
head c (exactly its GQA group) and W_out rows [256c : 256c+256).  Each core
computes a full-shape partial output; the host sums the 8 partials (the
unshard step for a row-sharded W_out).

On-core strategy:
  - all matmul operands are bf16 (PE runs 1 cycle/row vs 4 for fp32);
    accumulation stays fp32 in PSUM, softmax statistics stay fp32
  - x and the weights are converted to bf16 on the host, so DMA traffic
    is halved and no on-chip conversion pass is needed
  - scores are built TRANSPOSED (S^T[j,i] = k_j . q_i) so PV needs no
    attention-matrix transpose and the softmax denominator comes free
    via an extra ones-column in V
  - RMS-norm of q/k bounds |scores/8| <= 8, so exp() without
    max-subtraction is safe; masked entries are zeroed after exp by
    multiplying with precomputed keep-masks (causal edge tiles dedupe
    to 4 patterns)
  - phase 2 runs i-block outer so denominators + out-projection for
    early token blocks overlap later attention; per (head, iblock) all
    QK matmuls issue before the PV accumulation so exp latency hides
  - q/k norm+rope are batched as 5 "heads" (4 q + 1 k) with the q/k
    scales and rotate-half signs folded into host-precomputed tables
"""

import numpy as np
import ml_dtypes

BF16 = ml_dtypes.bfloat16

T = 2048
D = 2048
NUM_HEADS = 32
NUM_KV = 8
HD = 64
N_CORES = 8
H_LOC = NUM_HEADS // N_CORES  # 4 q heads per core
EPS = 1e-6

TT = T // 128   # 16 t-tiles of 128 rows
CC = D // 128   # 16 contraction chunks
IBS = T // 512  # 4 i-blocks of 512 query positions
JBS = T // 128  # 16 j-blocks of 128 key positions

KEEP = "keep"
SKIP = "skip"
AFFINE = "affine"  # kept for test.py compat; no longer produced


def _classify_mask(mask: np.ndarray):
    """Per (ib, jb) scoresT tile: KEEP / SKIP / ('pat', idx) with deduped
    multiplicative keep-masks in S^T (j, i) layout.  A causal mask yields
    just 4 distinct edge patterns."""
    keep = ~mask
    status = [[KEEP] * JBS for _ in range(IBS)]
    pat_index: dict[bytes, int] = {}
    pats: list[np.ndarray] = []
    for ib in range(IBS):
        for jb in range(JBS):
            sub = keep[ib * 512:(ib + 1) * 512, jb * 128:(jb + 1) * 128]
            if sub.all():
                status[ib][jb] = KEEP
            elif not sub.any():
                status[ib][jb] = SKIP
            else:
                key = sub.tobytes()
                if key not in pat_index:
                    pat_index[key] = len(pats)
                    pats.append(sub.T.astype(np.float32))  # [128 j, 512 i]
                status[ib][jb] = ("pat", pat_index[key])
    patterns = (
        np.stack(pats) if pats else np.zeros((1, 128, 512), dtype=np.float32)
    )
    # leading i-columns that are fully masked in each pattern: the score
    # matmul / exp / PV only need the live suffix
    prefixes = []
    for p in patterns:
        alive = p.any(axis=0)
        prefixes.append(int(alive.argmax()) if alive.any() else 512)
    return status, patterns, prefixes


def _split_multiwaits(nc):
    """walrus in this container accepts only ONE sync-wait per instruction;
    hoist extra waits onto preceding same-engine NoOps (program order on the
    engine queue preserves the gating)."""
    import bass_rust
    from concourse import mybir

    n_fixed = 0
    for fn in nc.m.functions:
        for bb in fn.blocks:
            out = []
            for ins in bb.instructions:
                si = ins.sync_info
                if si is not None and si.on_wait and len(si.on_wait) > 1:
                    waits = list(si.on_wait)
                    ups = list(si.on_update) if si.on_update else []
                    for k, w in enumerate(waits[:-1]):
                        nop = mybir.InstNoOp(
                            name=f"{ins.name}-wnop{k}", ins=[], outs=[]
                        )
                        nop.engine = ins.engine
                        nop.sync_info = bass_rust.SyncInfo(
                            on_wait=[w], on_update=[]
                        )
                        out.append(nop)
                    ins.sync_info = bass_rust.SyncInfo(
                        on_wait=[waits[-1]], on_update=ups
                    )
                    n_fixed += 1
                out.append(ins)
            bb.instructions = out
    return n_fixed


def _build_program(status, n_pat, pat_prefix):
    import concourse.bass as bass
    import concourse.mybir as mybir
    import concourse.tile as tile
    from concourse.masks import make_identity

    f32 = mybir.dt.float32
    bf16 = mybir.dt.bfloat16
    AX = mybir.AxisListType
    AF = mybir.ActivationFunctionType

    nc = bass.Bass("TRN2", num_devices=N_CORES)
    x_d = nc.declare_dram_parameter("x", [T, D], bf16, isOutput=False)
    wqkv_d = nc.declare_dram_parameter(
        "wqkv", [D, H_LOC * HD + 2 * HD], bf16, isOutput=False
    )
    wo_d = nc.declare_dram_parameter("wo", [H_LOC * HD, D], bf16, isOutput=False)
    # combined 5-unit rope tables: 4 q heads + k, scales folded in
    cosa_d = nc.declare_dram_parameter("cosa", [T, 5 * HD], bf16, isOutput=False)
    sina_d = nc.declare_dram_parameter("sina", [T, 5 * HD], bf16, isOutput=False)
    mpat_d = nc.declare_dram_parameter(
        "mpat", [n_pat, 128, 512], bf16, isOutput=False
    )
    out_d = nc.declare_dram_parameter("out", [T, D], bf16, isOutput=True)

    NQKV = H_LOC * HD + 2 * HD  # 384: q heads, then k, then v
    NQK = (H_LOC + 1) * HD      # 320: q heads + k (norm/rope batch)

    def mmr(out, lhsT, rhs, **kw):
        nc.tensor.matmul(out, lhsT, rhs, **kw)

    with tile.TileContext(nc) as tc:
        with (
            tc.tile_pool(name="const", bufs=1) as const,
            tc.tile_pool(name="persist", bufs=1) as persist,
        ):
            ident = const.tile([128, 128], bf16)
            make_identity(nc, ident)
            eps_t = const.tile([128, 1], f32)
            nc.vector.memset(eps_t, EPS)
            ones_t = const.tile([128, 64], bf16)
            nc.vector.memset(ones_t, 1.0)

            # persistent across phases (all bf16 matmul operands).
            # qT/kT hold only the real 64 head dims: score matmuls
            # contract K=64, so no zero-padding rows are needed.
            qkT = persist.tile([64, 5, T], bf16)
            # v with aux columns:
            #  v_aug  [128,TT,65]:  cols 0:64 = v, col 64 = 1  (even head of pair)
            #  v_aug2 [128,TT,128]: col 32 = 1, cols 64:128 = v (odd head of pair)
            v_aug = persist.tile([128, TT, 65], bf16)
            nc.vector.memset(v_aug[:, :, 64:65], 1.0)
            v_aug2 = persist.tile([128, TT, 128], bf16)
            nc.vector.memset(v_aug2[:, :, 0:64], 0.0)
            nc.vector.memset(v_aug2[:, :, 32:33], 1.0)
            ctxB = [persist.tile([128, T], bf16, name=f"ctxB{p}") for p in range(2)]
            dbc = [persist.tile([128, T], f32, name=f"dbc{p}") for p in range(2)]
            wo_sb = [persist.tile([128, D], bf16, name=f"wo{p}") for p in range(2)]
            for p in range(2):
                nc.gpsimd.dma_start(
                    out=wo_sb[p], in_=wo_d[p * 128:(p + 1) * 128, :]
                )
            mpat_sb = persist.tile([128, n_pat, 512], bf16, name="mpat_sb")
            nc.gpsimd.dma_start(
                out=mpat_sb, in_=mpat_d.rearrange("n p f -> p n f")
            )

            # ---------- phase 1: transpose x, project qkv, norm+rope ----------
            with (
                tc.tile_pool(name="p1w", bufs=1) as p1w,
                tc.tile_pool(name="p1s", bufs=3) as p1s,
                tc.tile_pool(name="p1t", bufs=3) as p1t,
                tc.tile_pool(name="ps1a", bufs=3, space="PSUM") as ps1a,
                tc.tile_pool(name="ps1b", bufs=2, space="PSUM") as ps1b,
                tc.tile_pool(name="ps1c", bufs=2, space="PSUM") as ps1c,
            ):
                # weight/table DMAs ride the DVE trigger queue so the
                # per-tt x loads on the sync queue start immediately
                wqkv_sb = p1w.tile([128, CC, NQKV], bf16)
                wqkv_r = wqkv_d.rearrange("(cc p) m -> p cc m", p=128)
                ctab = p1w.tile([128, TT, 5, HD], bf16, name="ctab")
                ctab_r = cosa_d.rearrange("(tt p) (u d) -> p tt u d", p=128, u=5)
                stab = p1w.tile([128, TT, 5, HD], bf16, name="stab")
                stab_r = sina_d.rearrange("(tt p) (u d) -> p tt u d", p=128, u=5)
                # chunked + interleaved so the first tiles' operands land
                # early instead of queueing behind 2.6 MB of tables
                for wc in range(4):
                    sl = slice(wc * 4, (wc + 1) * 4)
                    nc.scalar.dma_start(out=wqkv_sb[:, sl, :],
                                        in_=wqkv_r[:, sl, :])
                    nc.scalar.dma_start(out=ctab[:, sl], in_=ctab_r[:, sl])
                    nc.scalar.dma_start(out=stab[:, sl], in_=stab_r[:, sl])

                for tt in range(TT):
                    x_nat = p1s.tile([128, D], bf16, tag="x_nat")
                    nc.sync.dma_start(
                        out=x_nat, in_=x_d[tt * 128:(tt + 1) * 128, :]
                    )
                    xt_col = p1s.tile([128, CC, 128], bf16, tag="xt_col")
                    for cg in range(4):
                        pst = ps1a.tile([128, 512], bf16, tag="pst")
                        for k in range(4):
                            cc = cg * 4 + k
                            nc.tensor.transpose(
                                pst[:, k * 128:(k + 1) * 128],
                                x_nat[:, cc * 128:(cc + 1) * 128],
                                ident,
                            )
                        eng = nc.vector.tensor_copy if cg % 2 == 0 else nc.scalar.copy
                        eng(
                            xt_col[:, cg * 4:(cg + 1) * 4, :]
                            .rearrange("p a b -> p (a b)"),
                            pst,
                        )
                    psqkv = ps1b.tile([128, NQKV], f32, tag="psqkv")
                    for cc in range(CC):
                        mmr(psqkv, xt_col[:, cc, :], wqkv_sb[:, cc, :],
                            start=(cc == 0), stop=(cc == CC - 1))
                    psv = psqkv[:, NQK:NQKV]

                    nc.scalar.copy(v_aug[:, tt, 0:64], psv)
                    nc.scalar.copy(v_aug2[:, tt, 64:128], psv)

                    # rms-norm + rope for 4 q heads + k in one 5-unit batch
                    # (PSUM -> SBUF first: DVE tensor-tensor can't read PSUM)
                    qk5 = p1t.tile([128, 5, HD], f32, tag="qk5")
                    nc.scalar.copy(
                        qk5, psqkv[:, 0:NQK].rearrange("p (u d) -> p u d", u=5)
                    )
                    sq = p1t.tile([128, 5, HD], f32, tag="sq")
                    nc.scalar.activation(
                        sq, psqkv[:, 0:NQK].rearrange("p (u d) -> p u d", u=5),
                        AF.Square,
                    )
                    ssum = p1t.tile([128, 5, 1], f32, tag="ssum")
                    nc.vector.reduce_sum(ssum, sq, axis=AX.X)
                    rinv = p1t.tile([128, 5, 1], f32, tag="rinv")
                    nc.scalar.activation(rinv, ssum, AF.Sqrt,
                                         bias=eps_t[:, 0:1], scale=1.0 / HD)
                    nc.vector.reciprocal(rinv, rinv)
                    qn = p1t.tile([128, 5, HD], bf16, tag="qn")
                    nc.vector.tensor_mul(
                        qn, qk5, rinv.to_broadcast([128, 5, HD])
                    )
                    qr = p1t.tile([128, 5, HD], bf16, tag="qr")
                    nc.vector.tensor_mul(qr, qn, ctab[:, tt, :, :])
                    qrot = p1t.tile([128, 5, HD], bf16, tag="qrot")
                    nc.gpsimd.tensor_mul(
                        qrot[:, :, 0:32], qn[:, :, 32:64],
                        stab[:, tt, :, 0:32],
                    )
                    nc.gpsimd.tensor_mul(
                        qrot[:, :, 32:64], qn[:, :, 0:32],
                        stab[:, tt, :, 32:64],
                    )
                    qb = p1t.tile([128, 5, HD], bf16, tag="qb")
                    nc.vector.tensor_add(qb, qr, qrot)

                    # transpose the 5 units into qT / kT
                    psqt = ps1c.tile([64, 5, 128], bf16, tag="psqt")
                    for u in range(5):
                        nc.tensor.transpose(psqt[:, u, :], qb[:, u, :], ident)
                    nc.vector.tensor_copy(
                        qkT[:, :, tt * 128:(tt + 1) * 128], psqt
                    )

            # ---------- phase 2: attention + denominators + out-proj ----------
            with (
                tc.tile_pool(name="p2e", bufs=8) as p2e,
                tc.tile_pool(name="ps2s", bufs=4, space="PSUM") as ps2s,
                tc.tile_pool(name="ps2c", bufs=2, space="PSUM") as ps2c,
                tc.tile_pool(name="ps2o", bufs=2, space="PSUM") as ps2o,
            ):

                inv_sqrt_d = float(1.0 / np.sqrt(HD))

                def out_proj_quarter(ib, t4):
                    tt = ib * 4 + t4
                    for cb in range(4):
                        pso = ps2o.tile([128, 512], f32, tag="pso")
                        for pair in range(2):
                            mmr(pso,
                                ctxB[pair][:, tt * 128:(tt + 1) * 128],
                                wo_sb[pair][:, cb * 512:(cb + 1) * 512],
                                start=(pair == 0), stop=(pair == 1))
                        ot = p2e.tile([128, 512], bf16, tag="ot")
                        nc.vector.tensor_copy(ot, pso)
                        nc.sync.dma_start(
                            out=out_d[tt * 128:(tt + 1) * 128,
                                      cb * 512:(cb + 1) * 512],
                            in_=ot,
                        )

                def den_pair(ib, pair, pe, po):
                    # denominators: reciprocal (bf16) into SBUF staging at
                    # the same partition rows, then broadcast across the
                    # partition dim with a K=1 ones-matmul (out reuses a
                    # pss ring slot), stage to SBUF, scale ctx into ctxB
                    iw = slice(ib * 512, (ib + 1) * 512)
                    den_sb = p2e.tile([65, 512], bf16, tag="den_sb")
                    with nc.allow_low_precision(
                        reason="1/den in bf16: 0.4% on softmax scale is "
                               "well inside the 2e-2 tolerance"
                    ):
                        nc.vector.reciprocal(den_sb[64:65, :], pe[64:65, :])
                        nc.vector.reciprocal(den_sb[32:33, :], po[32:33, :])
                    pdb = ps2s.tile([128, 512], f32, tag="pss")
                    mmr(pdb[0:64, :], ones_t[64:65, :], den_sb[64:65, :],
                        start=True, stop=True)
                    mmr(pdb[64:128, :], ones_t[32:33, :], den_sb[32:33, :],
                        start=True, stop=True)
                    nc.vector.tensor_copy(dbc[pair][:, iw], pdb)
                    # stage ctx PSUM -> SBUF (only DVE/ACT read PSUM),
                    # then scale by 1/den on DVE into bf16
                    ctx_s = p2e.tile([128, 512], f32, tag="ctx_s")
                    nc.vector.tensor_copy(ctx_s[0:64, :], pe[0:64, :])
                    nc.vector.tensor_copy(ctx_s[64:128, :], po[64:128, :])
                    nc.vector.tensor_mul(
                        ctxB[pair][:, iw], ctx_s, dbc[pair][:, iw],
                    )

                for ib in range(IBS):
                    iw = slice(ib * 512, (ib + 1) * 512)
                    jbs = [jb for jb in range(JBS) if status[ib][jb] != SKIP]
                    psc_of = {}
                    for h in range(H_LOC):
                        pair, sub = divmod(h, 2)
                        psc = ps2c.tile([128, 512], f32, tag="psc")
                        psc_of[h] = psc
                        if sub == 0:
                            ctx_out = psc[0:65, :]
                            lhs_of = lambda jb: v_aug[:, jb, :]
                        else:
                            ctx_out = psc
                            lhs_of = lambda jb: v_aug2[:, jb, :]
                        # all QK matmuls first; exp/mask trail on ACT/DVE;
                        # then the PV accumulation chain (never stalls PE)
                        ets = []
                        for jb in jbs:
                            st = status[ib][jb]
                            pre = (pat_prefix[st[1]]
                                   if isinstance(st, tuple) else 0)
                            cw = slice(pre, 512)
                            pss = ps2s.tile([128, 512], f32, tag="pss")
                            mmr(pss[:, cw],
                                qkT[:, 4, jb * 128:(jb + 1) * 128],
                                qkT[:, h, ib * 512 + pre:(ib + 1) * 512],
                                start=True, stop=True)
                            et = p2e.tile([128, 512], bf16, tag="et")
                            nc.scalar.activation(et[:, cw], pss[:, cw],
                                                 AF.Exp, scale=inv_sqrt_d)
                            if isinstance(st, tuple):
                                nc.vector.tensor_mul(
                                    et[:, cw], et[:, cw],
                                    mpat_sb[:, st[1], cw]
                                )
                            ets.append((et, cw))
                        for n, jb in enumerate(jbs):
                            et, cw = ets[n]
                            mmr(ctx_out[:, cw], lhs_of(jb), et[:, cw],
                                start=(n == 0), stop=(n == len(jbs) - 1))
                        # denominator bounce starts as soon as a pair's
                        # two heads are done, overlapping the next heads
                        if sub == 1:
                            den_pair(ib, pair, psc_of[h - 1], psc)
                            # out-projection lags one i-block behind; its
                            # matmuls fill PE under this block's exp waits
                            if ib > 0:
                                out_proj_quarter(ib - 1, 2 * pair)
                                out_proj_quarter(ib - 1, 2 * pair + 1)
                for t4 in range(4):
                    out_proj_quarter(IBS - 1, t4)

    _split_multiwaits(nc)
    return nc


_CACHE = {}


def _get_program(mask_key, status, n_pat, pat_prefix):
    if mask_key not in _CACHE:
        _CACHE[mask_key] = _build_program(status, n_pat, pat_prefix)
    return _CACHE[mask_key]


def _prepare(x, mask, cos, sin, W_query, W_key, W_value, W_out,
             q_scale, k_scale):
    """Host-side prep: fold scales into rope tables, shard weights,
    classify the mask.  Returns (nc, in_maps)."""
    cos = np.asarray(cos, dtype=np.float32)
    sin = np.asarray(sin, dtype=np.float32)
    W_query = np.asarray(W_query, dtype=np.float32)
    W_key = np.asarray(W_key, dtype=np.float32)
    W_value = np.asarray(W_value, dtype=np.float32)
    W_out = np.asarray(W_out, dtype=np.float32)
    q_scale = np.asarray(q_scale, dtype=np.float32)
    k_scale = np.asarray(k_scale, dtype=np.float32)
    mask = np.asarray(mask)

    xf = np.ascontiguousarray(
        np.asarray(x).reshape(T, D).astype(BF16)
    )

    # rope = qn*cos' + shuffle32(qn)*sin' with the rotate-half signs and the
    # post-norm q/k scales folded into the tables:
    #   rope(s*qn) = qn*(s*cos) + shuffle32(qn)*(shuffle32(s)*sin+-)
    def tables(scale):
        perm = np.concatenate([scale[HD // 2:], scale[:HD // 2]])
        c = (cos * scale[None, :]).astype(np.float32)
        s = (sin * perm[None, :]).astype(np.float32)
        s[:, :HD // 2] *= -1.0
        return c, s

    cq, sq_t = tables(q_scale)
    ck, sk_t = tables(k_scale)
    # 5-unit tables: 4 q heads then k  -> [T, 320]
    cosa = np.ascontiguousarray(
        np.concatenate([cq, cq, cq, cq, ck], axis=1).astype(BF16)
    )
    sina = np.ascontiguousarray(
        np.concatenate([sq_t, sq_t, sq_t, sq_t, sk_t], axis=1).astype(BF16)
    )

    status, patterns, prefixes = _classify_mask(mask)
    nc = _get_program(mask.tobytes(), status, patterns.shape[0], prefixes)
    patterns_bf = patterns.astype(BF16)

    in_maps = []
    for c in range(N_CORES):
        qcols = slice(c * H_LOC * HD, (c + 1) * H_LOC * HD)
        kvcols = slice(c * HD, (c + 1) * HD)
        wqkv = np.concatenate(
            [W_query[:, qcols], W_key[:, kvcols], W_value[:, kvcols]], axis=1
        ).astype(BF16)
        in_maps.append({
            "x": xf,
            "wqkv": np.ascontiguousarray(wqkv),
            "wo": np.ascontiguousarray(W_out[qcols, :].astype(BF16)),
            "cosa": cosa, "sina": sina,
            "mpat": patterns_bf,
        })
    return nc, in_maps


def kernel(x, mask, cos, sin, W_query, W_key, W_value, W_out,
           q_scale, k_scale):
    out_dtype = np.asarray(x).dtype
    nc, in_maps = _prepare(x, mask, cos, sin, W_query, W_key, W_value,
                           W_out, q_scale, k_scale)

    from concourse.bass_utils import run_bass_kernel_spmd

    res = run_bass_kernel_spmd(nc, in_maps, list(range(N_CORES)))
    acc = res.results[0]["out"].astype(np.float32)
    for c in range(1, N_CORES):
        acc = acc + res.results[c]["out"].astype(np.float32)
    return acc.reshape(1, T, D).astype(out_dtype)
